# revision 14
# baseline (speedup 1.0000x reference)
"""TRN2 Bass/Tile kernel: nn_ChannelWiseTensorSquareSelfInteraction.

Contract: kernel(**inputs) takes the FULL unsharded inputs
(x [100000,512], mlp_w1 [384,384], mlp_w2 [384,768], lin_ws [384,128],
lin_wv [256,128], all fp32) and returns the FULL output [100000,512] fp32.

Strategy (8 NeuronCores, data-parallel over the node axis):
  - Host: pad nodes 100000 -> 8*12800, shard; de-interleave x into
    feature-major chunks [s | vx | vy | vz] each [128, nodes] so the
    device kernel needs no input transposes. Weights replicated; the
    sqrt(2) factor of the sv path and the unused 2e-gate columns of
    mlp_w2 are folded/dropped host-side.
  - Device (per core, feature-major: channels on partitions, nodes on
    the free dim, tiles of 512 nodes):
      products ss/vv (GPSIMD+DVE) -> mm1+silu -> mm2+silu (PE+ACT,
      float32r matmuls at full PE rate) -> gating muls (DVE/GPSIMD) ->
      equivariant linear with the residual folded in as an
      identity-matmul accumulate (PE) -> PE transpose back to
      node-major -> LayerNorm fused with PSUM evacuation (DVE/ACT) ->
      store node-major (the 1o interleave restored via a strided AP).
"""

import numpy as np

import concourse.bacc as bacc
import concourse.mybir as mybir
from concourse.tile import TileContext
from concourse.masks import make_identity
from concourse.bass_utils import run_bass_kernel_spmd

F32 = mybir.dt.float32
F32R = mybir.dt.float32r
AF = mybir.ActivationFunctionType
OP = mybir.AluOpType
EPS = 1e-6

N_FULL = 100000
N_CORES = 8
NPC = 12800  # padded nodes per core (25 tiles of 512)

# engine/pool tuning knobs (overridable for cost-model experiments)
OPTS = dict(
    ph_bufs=1, pg_bufs=2, po_bufs=1, pnm_bufs=4,
    conv_engine="act",      # producer of s_r/ss_r/vv_r copies: act|gps|dve
    gsv_engine="dve",       # gsv mul: dve|gps
    vg_split=0,             # how many of the 3 vg muls go to gps
)


def build_nc(npc: int = NPC, T: int = 512):
    """Build the per-core Bass program. npc = nodes per core."""
    assert npc % T == 0 and T % 128 == 0
    NT = npc // T
    NB = T // 128  # 128-node blocks per tile

    nc = bacc.Bacc("TRN2", target_bir_lowering=False, debug=False, num_devices=N_CORES)
    xt = nc.declare_dram_parameter("xt", [4, 128, npc], F32, isOutput=False)
    w1 = nc.declare_dram_parameter("w1", [384, 384], F32R, isOutput=False)
    w2 = nc.declare_dram_parameter("w2", [384, 640], F32R, isOutput=False)
    ws = nc.declare_dram_parameter("ws", [384, 128], F32R, isOutput=False)
    wv = nc.declare_dram_parameter("wv", [256, 128], F32R, isOutput=False)
    xs_r = nc.declare_dram_parameter("xs_r", [128, npc], F32R, isOutput=False)
    y = nc.declare_dram_parameter("y", [npc, 512], F32, isOutput=True)

    xt_r = xt.rearrange("c p n -> p c n")
    y_r = y.rearrange("(t b p) f -> t p b f", b=NB, p=128)

    with TileContext(nc) as tc:
        with (
            tc.tile_pool(name="singles", bufs=1) as singles,
            tc.tile_pool(name="pin", bufs=OPTS.get("pin_bufs",3)) as pin,
            tc.tile_pool(name="pmid", bufs=OPTS.get("pmid_bufs",2)) as pmid,
            tc.tile_pool(name="pout", bufs=OPTS.get("pout_bufs",2)) as pout,
            tc.tile_pool(name="psmall", bufs=OPTS.get("psmall_bufs",3)) as psmall,
            tc.tile_pool(name="ph", bufs=OPTS["ph_bufs"], space="PSUM") as ph,
            tc.tile_pool(name="pg", bufs=OPTS["pg_bufs"], space="PSUM") as pg,
            tc.tile_pool(name="po", bufs=OPTS["po_bufs"], space="PSUM") as po,
            tc.tile_pool(name="pnm", bufs=OPTS["pnm_bufs"], space="PSUM") as pnm,
        ):
            # --- resident weights (host pre-rounded to the PE f32r format) ---
            w1_r = singles.tile([128, 3, 384], F32R)
            nc.sync.dma_start(out=w1_r, in_=w1.rearrange("(k p) m -> p k m", p=128))
            w2_r = singles.tile([128, 3, 640], F32R)
            nc.sync.dma_start(out=w2_r, in_=w2.rearrange("(k p) m -> p k m", p=128))
            ws_r = singles.tile([128, 3, 128], F32R)
            nc.sync.dma_start(out=ws_r, in_=ws.rearrange("(k p) m -> p k m", p=128))
            wv_r = singles.tile([128, 2, 128], F32R)
            nc.sync.dma_start(out=wv_r, in_=wv.rearrange("(k p) m -> p k m", p=128))
            ident = singles.tile([128, 128], F32)
            make_identity(nc, ident)

            def mm(out_p, lhsT, rhs, start, stop):
                nc.tensor.matmul(out_p, lhsT, rhs, start=start, stop=stop)

            for t in range(NT):
                ns = slice(t * T, (t + 1) * T)
                def conv(dst, srcv):
                    if OPTS["conv_engine"] == "act":
                        nc.scalar.activation(out=dst, in_=srcv, func=AF.Copy)
                    elif OPTS["conv_engine"] == "gps":
                        nc.gpsimd.tensor_copy(out=dst, in_=srcv)
                    else:
                        nc.vector.tensor_copy(out=dst, in_=srcv)

                xin = pin.tile([128, 4, T], F32, tag="xin")
                nc.sync.dma_start(out=xin, in_=xt_r[:, :, ns])
                s = xin[:, 0, :]
                v3 = [xin[:, 1, :], xin[:, 2, :], xin[:, 3, :]]

                # --- channel-wise products (scal chunks; f32r copies for the PE) ---
                def _conv_unused(dst, srcv):
                    if OPTS["conv_engine"] == "act":
                        nc.scalar.activation(out=dst, in_=srcv, func=AF.Copy)
                    elif OPTS["conv_engine"] == "gps":
                        nc.gpsimd.tensor_copy(out=dst, in_=srcv)
                    else:
                        nc.vector.tensor_copy(out=dst, in_=srcv)

                s_r = pmid.tile([128, T], F32R, tag="s_r")
                if OPTS.get("sr_dma", False):
                    nc.sync.dma_start(out=s_r, in_=xs_r[:, ns])
                else:
                    conv(s_r, s)
                ss_f = pmid.tile([128, T], F32, tag="ss_f")
                nc.gpsimd.tensor_mul(ss_f, s, s)
                ss_r = pmid.tile([128, T], F32R, tag="ss_r")
                conv(ss_r, ss_f)
                sqx = pmid.tile([128, T], F32, tag="sqx")
                nc.gpsimd.tensor_mul(sqx, v3[0], v3[0])
                sqy = pmid.tile([128, T], F32, tag="sqy")
                nc.gpsimd.tensor_mul(sqy, v3[1], v3[1])
                sqz = pmid.tile([128, T], F32, tag="sqz")
                nc.gpsimd.tensor_mul(sqz, v3[2], v3[2])
                vvp = pmid.tile([128, T], F32, tag="vvp")
                nc.vector.tensor_add(vvp, sqx, sqy)
                vv_f = pmid.tile([128, T], F32, tag="vv_f")
                nc.vector.tensor_add(vv_f, vvp, sqz)
                vv_r = pmid.tile([128, T], F32R, tag="vv_r")
                conv(vv_r, vv_f)
                scal_f = [s, ss_f, vv_f]
                scal_r = [s_r, ss_r, vv_r]

                # --- MLP layer 1: hidden = silu(scal @ w1) ---
                h_sb = pmid.tile([128, 3, T], F32R, tag="h")
                for m in range(3):
                    psum_h = ph.tile([128, T], F32, tag="ph")
                    for k in range(3):
                        mm(psum_h, w1_r[:, k, 128 * m : 128 * (m + 1)], scal_r[k],
                           start=(k == 0), stop=(k == 2))
                    nc.scalar.activation(out=h_sb[:, m, :], in_=psum_h, func=AF.Silu)

                # --- MLP layer 2: gates = silu(hidden @ w2[:, :640]) ---
                g_sb = pmid.tile([128, 5, T], F32, tag="g")
                for m in range(5):
                    psum_g = pg.tile([128, T], F32, tag="pg")
                    for k in range(3):
                        mm(psum_g, w2_r[:, k, 128 * m : 128 * (m + 1)], h_sb[:, k, :],
                           start=(k == 0), stop=(k == 2))
                    nc.scalar.activation(out=g_sb[:, m, :], in_=psum_g, func=AF.Silu)
                gv1 = g_sb[:, 3, :]
                gv2 = g_sb[:, 4, :]

                # --- gating (elementwise, f32r outputs feed the PE) ---
                sg = pmid.tile([128, 3, T], F32R, tag="sg")
                nc.vector.tensor_mul(sg[:, 0, :], scal_f[0], g_sb[:, 0, :])
                nc.vector.tensor_mul(sg[:, 1, :], scal_f[1], g_sb[:, 1, :])
                nc.gpsimd.tensor_mul(sg[:, 2, :], scal_f[2], g_sb[:, 2, :])
                vg = pmid.tile([128, 3, T], F32R, tag="vg")
                for i in range(3):
                    eng = nc.gpsimd if i < OPTS["vg_split"] else nc.vector
                    eng.tensor_mul(vg[:, i, :], v3[i], gv1)
                gsv = pmid.tile([128, T], F32, tag="gsv")
                (nc.gpsimd if OPTS["gsv_engine"] == "gps" else nc.vector).tensor_mul(gsv, s, gv2)
                svg = pmid.tile([128, 3, T], F32R, tag="svg")
                for i in range(3):
                    nc.gpsimd.tensor_mul(svg[:, i, :], gsv, v3[i])

                # --- equivariant linear; residual added exactly during evacuation ---
                oc_sb = pout.tile([128, 4, T], F32, tag="oc")
                pos = po.tile([128, T], F32, tag="po")
                for k in range(3):
                    mm(pos, ws_r[:, k, :], sg[:, k, :], start=(k == 0), stop=(k == 2))
                nc.vector.scalar_tensor_tensor(
                    out=oc_sb[:, 0, :], in0=pos, scalar=0.0, in1=s,
                    op0=OP.add, op1=OP.add,
                )
                for i in range(3):
                    pov = po.tile([128, T], F32, tag="po")
                    mm(pov, wv_r[:, 0, :], vg[:, i, :], start=True, stop=False)
                    mm(pov, wv_r[:, 1, :], svg[:, i, :], start=False, stop=True)
                    nc.vector.scalar_tensor_tensor(
                        out=oc_sb[:, 1 + i, :], in0=pov, scalar=0.0, in1=v3[i],
                        op0=OP.add, op1=OP.add,
                    )

                # --- transpose to node-major + LayerNorm + store ---
                # sq[:, 0, :] holds 128*var_s per block; sq[:, 1, :] holds
                # sum(v^2) per block. One batched Newton-rsqrt chain then
                # yields 1/(sqrt(var)+eps) for both parts.
                y_sb = pout.tile([128, NB, 512], F32, tag="y")
                stats = psmall.tile([128, NB, 6], F32, tag="stats")
                mv = psmall.tile([128, NB, 2], F32, tag="mv")
                sq = psmall.tile([128, 2, NB], F32, tag="sq")
                pnm_ts = []
                for b in range(NB):
                    pnm_t = pnm.tile([128, 512], F32, tag="pnm")
                    pnm_ts.append(pnm_t)
                    for c in range(4):
                        nc.tensor.matmul(
                            pnm_t[:, 128 * c : 128 * (c + 1)],
                            oc_sb[:, c, 128 * b : 128 * (b + 1)],
                            ident,
                            is_transpose=True,
                        )
                    # scalar-part stats: mean/var over the 128 features
                    nc.vector.bn_stats(out=stats[:, b, :], in_=pnm_t[:, 0:128])
                    nc.vector.bn_aggr(out=mv[:, b, :], in_=stats[:, b, :])
                    # vector-part sumsq over all 384 components
                    vscr = pmid.tile([128, 384], F32, tag="vscr")
                    nc.scalar.activation(
                        out=vscr, in_=pnm_t[:, 128:512], func=AF.Square,
                        accum_out=sq[:, 1, b : b + 1],
                    )
                # gather 128*var_s (both halves of sq then hold 128*mean-square)
                nc.vector.tensor_scalar(
                    out=sq[:, 0, :], in0=mv[:, :, 1], scalar1=128.0, scalar2=None,
                    op0=OP.mult,
                )
                # inv = 1/(sqrt(w/128)+eps) via Newton rsqrt (keeps ACT on one
                # table set; exact to ~1e-5): seed = magic - (bits>>1), two
                # iterations y *= 1.5 - 0.5*w*y^2, then d=w*y, inv=1/(d+eps).
                w = psmall.tile([128, 2 * NB], F32, tag="nw")
                nc.vector.tensor_scalar(
                    out=w, in0=sq.rearrange("p a b -> p (a b)"),
                    scalar1=1.0 / 128.0, scalar2=None, op0=OP.mult,
                )
                wi = w.bitcast(mybir.dt.int32)
                yv = psmall.tile([128, 2 * NB], F32, tag="ny")
                yi = yv.bitcast(mybir.dt.int32)
                nc.vector.tensor_scalar(out=yi, in0=wi, scalar1=1, scalar2=None,
                                        op0=OP.arith_shift_right)
                nc.vector.tensor_scalar(out=yi, in0=yi, scalar1=0x5F3759E0,
                                        scalar2=None, op0=OP.subtract)
                nc.vector.tensor_scalar(out=yi, in0=yi, scalar1=-1, scalar2=None,
                                        op0=OP.bitwise_xor)
                hv = psmall.tile([128, 2 * NB], F32, tag="nh")
                nc.vector.tensor_scalar(out=hv, in0=w, scalar1=0.5, scalar2=None,
                                        op0=OP.mult)
                tmp = psmall.tile([128, 2 * NB], F32, tag="nt")
                for _ in range(2):
                    nc.vector.tensor_mul(tmp, yv, yv)
                    nc.vector.tensor_mul(tmp, tmp, hv)
                    nc.vector.tensor_scalar(out=tmp, in0=tmp, scalar1=-1.0,
                                            scalar2=1.5, op0=OP.mult, op1=OP.add)
                    nc.vector.tensor_mul(yv, yv, tmp)
                den = psmall.tile([128, 2 * NB], F32, tag="nd")
                nc.vector.tensor_mul(den, w, yv)
                nc.vector.tensor_scalar_add(den, den, EPS)
                inv = psmall.tile([128, 2 * NB], F32, tag="ninv")
                nc.vector.reciprocal(inv, den)
                # bias for the scalar part: -mu * inv_s
                nbias = psmall.tile([128, NB], F32, tag="nbias")
                nc.vector.scalar_tensor_tensor(
                    out=nbias, in0=mv[:, :, 0], scalar=-1.0, in1=inv[:, 0:NB],
                    op0=OP.mult, op1=OP.mult,
                )
                for b in range(NB):
                    pnm_t = pnm_ts[b]
                    # normalize on ACT: out = in*scale + bias (per-node scalars)
                    nc.scalar.activation(
                        out=y_sb[:, b, 0:128], in_=pnm_t[:, 0:128], func=AF.Identity,
                        bias=nbias[:, b : b + 1], scale=inv[:, b : b + 1],
                    )
                    vdst = y_sb[:, b, 128:512].rearrange("p (o i) -> p i o", i=3)
                    vsrc = pnm_t[:, 128:512].rearrange("p (i o) -> p i o", o=128)
                    if b < OPTS.get("tsv_act", 0):
                        nc.scalar.activation(
                            out=vdst, in_=vsrc, func=AF.Copy,
                            scale=inv[:, NB + b : NB + b + 1],
                        )
                    else:
                        nc.vector.tensor_scalar(
                            out=vdst, in0=vsrc,
                            scalar1=inv[:, NB + b : NB + b + 1], scalar2=None,
                            op0=OP.mult,
                        )
                if OPTS.get("out_dma_scalar", False):
                    nc.scalar.dma_start(out=y_r[t], in_=y_sb)
                else:
                    nc.sync.dma_start(out=y_r[t], in_=y_sb)

    nc.finalize()
    return nc


def _round_f32r(a):
    """Round fp32 to the PE's f32r precision (11 explicit mantissa bits)."""
    i = np.ascontiguousarray(a, np.float32).view(np.int32)
    r = ((i + 0x7FF + ((i >> 12) & 1)) >> 12) << 12
    return r.astype(np.int32).view(np.float32)


def host_prep(x_full, mlp_w1, mlp_w2, lin_ws, lin_wv, npc: int = NPC):
    """Pad + shard + feature-major de-interleave. Returns 8 input maps."""
    x_full = np.asarray(x_full, np.float32)
    n = x_full.shape[0]
    xp = np.zeros((N_CORES * npc, 512), dtype=np.float32)
    xp[:n] = x_full
    w1 = _round_f32r(np.asarray(mlp_w1, np.float32))
    w2 = _round_f32r(np.asarray(mlp_w2, np.float32)[:, :640])
    ws_ = _round_f32r(np.asarray(lin_ws, np.float32))
    wv_np = np.asarray(lin_wv, np.float32)
    wv_ = _round_f32r(
        np.concatenate([wv_np[:128], np.float32(np.sqrt(2.0)) * wv_np[128:]], axis=0)
    )
    maps = []
    for c in range(N_CORES):
        xs = xp[c * npc : (c + 1) * npc]
        xtc = np.empty((4, 128, npc), dtype=np.float32)
        xtc[0] = xs[:, :128].T
        v = xs[:, 128:].reshape(npc, 128, 3)
        xtc[1] = v[:, :, 0].T
        xtc[2] = v[:, :, 1].T
        xtc[3] = v[:, :, 2].T
        maps.append(dict(xt=xtc, w1=w1, w2=w2, ws=ws_, wv=wv_,
                         xs_r=_round_f32r(xtc[0])))
    return maps


_CACHE = {}


def _get_nc():
    if "nc" not in _CACHE:
        _CACHE["nc"] = build_nc()
    return _CACHE["nc"]


def kernel(x, mlp_w1, mlp_w2, lin_ws, lin_wv):
    maps = host_prep(x, mlp_w1, mlp_w2, lin_ws, lin_wv)
    nc = _get_nc()
    res = run_bass_kernel_spmd(nc, maps, list(range(N_CORES)))
    n = np.asarray(x).shape[0]
    out = np.concatenate([res.results[c]["y"] for c in range(N_CORES)], axis=0)[:n]
    return np.ascontiguousarray(out)


def timed_stats():
    """Extra (test-only) instrumentation: simulated per-core exec time."""
    try:
        from concourse.timeline_sim import TimelineSim

        sim = TimelineSim(_get_nc())
        return float(sim.simulate())
    except Exception as e:  # pragma: no cover
        print("timeline sim failed:", e)
        return None
